# revision 1
# baseline (speedup 1.0000x reference)
"""Per-pixel adaptive 5x5 conv (KPN) for Trainium2, 8-core data parallel.

out[g,h,w] = sum_{i,j} core[g,5i+j,h,w] * frames_pad[g,h+i-2,w+j-2]
with g = flattened (B,N) = 16 image planes; 2 planes per NeuronCore.

Host prep builds DMA-friendly layouts (all fp16):
  fin [2, 128, 5*2*4*518]: per partition p: [i-shift:5][parity:2][blk:4][518]
     fprep[img,p,i,par,blk,c] = Fpad[img, blk*128+p+i, (1-par)+c]
     parity copies keep every tap's 512-col slice 4-byte aligned so the
     DVE 2x fp16 mode engages for all 25 (i,j) taps.
  win [2, 5, 128, 5*4*512]: tap-group-major core weights
     wprep[img,tg,p,k,blk,c] = core[img, 5*tg+k, blk*128+p, c]
On chip per image: 1 frames DMA + 5 weight-group DMAs; 25 taps of
mul+add at FD=2048 (4 row-blocks fused per op), 19 taps on DVE (fp16
2x mode) and 6 on GpSimd; two accumulator chains merged at the end;
fp16->fp32 cast on the output DMA (SWDGE).
"""

import os
import sys

import numpy as np

for _p in ("/opt/trn_rl_repo",):
    if _p not in sys.path and os.path.isdir(_p):
        sys.path.insert(0, _p)

K = 5
NCORES = 8
IMGS_PER_CORE = 2
H = W = 512
NBLK = 4  # 128-row blocks per image
FCOLS = 518
F_FREE = K * 2 * NBLK * FCOLS  # 20720
W_FREE = K * NBLK * W  # 10240
O_FREE = NBLK * W  # 2048

# gpsimd gets ~1/4 of taps (it runs 2-input elemwise ~2.9x slower than
# DVE fp16-2x): one tap per 5-tap group + one extra.
GP_TAPS = frozenset({4, 9, 14, 19, 23, 24})

_compiled = {}
last_results = None  # BassKernelResults of the most recent run (for test.py)


def _build_nc():
    import concourse.bacc as bacc
    import concourse.mybir as mybir
    from concourse.tile import TileContext

    f16 = mybir.dt.float16
    f32 = mybir.dt.float32

    nc = bacc.Bacc(None, target_bir_lowering=False, debug=False)
    fin = nc.dram_tensor("fin", [IMGS_PER_CORE, 128, F_FREE], f16,
                         kind="ExternalInput")
    win = nc.dram_tensor("win", [IMGS_PER_CORE, K, 128, W_FREE], f16,
                         kind="ExternalInput")
    oout = nc.dram_tensor("oout", [IMGS_PER_CORE, 128, O_FREE], f32,
                          kind="ExternalOutput")

    with TileContext(nc) as tc:
        with (
            tc.tile_pool(name="fpool", bufs=2) as fpool,
            tc.tile_pool(name="wpool", bufs=3) as wpool,
            tc.tile_pool(name="apool", bufs=2) as apool,
            tc.tile_pool(name="tpool", bufs=4) as tpool,
        ):
            FI = 2 * NBLK * FCOLS  # free elems per i-shift slice
            for img in range(IMGS_PER_CORE):
                # one tile+DMA per i-shift so taps of i=0 start after ~1MB
                fvs = []
                for i in range(K):
                    f_t = fpool.tile([128, FI], f16, tag=f"f{i}")
                    nc.sync.dma_start(out=f_t[:],
                                      in_=fin[img][:, i * FI:(i + 1) * FI])
                    fvs.append(f_t[:].rearrange(
                        "p (par blk c) -> p par blk c",
                        par=2, blk=NBLK, c=FCOLS))

                acc_v = apool.tile([128, O_FREE], f16, tag="accv")
                acc_g = apool.tile([128, O_FREE], f16, tag="accg")
                av = acc_v[:].rearrange("p (blk c) -> p blk c", blk=NBLK)
                ag = acc_g[:].rearrange("p (blk c) -> p blk c", blk=NBLK)
                first = {id(av): True, id(ag): True}

                for tg in range(K):
                    w_t = wpool.tile([128, W_FREE], f16)
                    nc.sync.dma_start(out=w_t[:], in_=win[img, tg])
                    wv = w_t[:].rearrange("p (k blk c) -> p k blk c",
                                          k=K, blk=NBLK, c=W)
                    # gpsimd taps first: it has the worst per-op latency
                    korder = sorted(range(K),
                                    key=lambda k: tg * K + k not in GP_TAPS)
                    for k in korder:
                        t = tg * K + k
                        i, j = divmod(t, K)
                        par = j & 1
                        joff = j + par
                        f_ap = fvs[i][:, par, :, joff:joff + W]
                        w_ap = wv[:, k]
                        if t in GP_TAPS:
                            eng, acc, tag = nc.gpsimd, ag, "tmpg"
                        else:
                            eng, acc, tag = nc.vector, av, "tmpv"
                        if first[id(acc)]:
                            eng.tensor_mul(out=acc, in0=w_ap, in1=f_ap)
                            first[id(acc)] = False
                        else:
                            tmp = tpool.tile([128, O_FREE], f16, tag=tag)
                            tv = tmp[:].rearrange("p (blk c) -> p blk c",
                                                  blk=NBLK)
                            eng.tensor_mul(out=tv, in0=w_ap, in1=f_ap)
                            eng.tensor_add(out=acc, in0=acc, in1=tv)

                nc.vector.tensor_add(out=acc_v[:], in0=acc_v[:], in1=acc_g[:])
                # SWDGE cast fp16 -> fp32 on the store
                nc.gpsimd.dma_start(out=oout[img], in_=acc_v[:])
    nc.finalize()
    return nc


def _host_prep(frames, core):
    """Build per-core in_maps. frames [4,4,1,512,512] f32, core [4,4,25,1,512,512]."""
    G = NCORES * IMGS_PER_CORE  # 16
    F = np.ascontiguousarray(frames.reshape(G, H, W))
    Wc = core.reshape(G, K * K, H, W)

    # frames: pad rows 2/2, cols 3/4; Fp[g, r, c] = F[g, r-2, c-3]
    Fp = np.pad(F, ((0, 0), (2, 2), (3, 4))).astype(np.float16)
    fprep = np.empty((G, 128, K, 2, NBLK, FCOLS), np.float16)
    for i in range(K):
        for par in range(2):
            sl = Fp[:, i:i + H, (1 - par):(1 - par) + FCOLS]  # [G,512,518]
            fprep[:, :, i, par, :, :] = (
                sl.reshape(G, NBLK, 128, FCOLS).transpose(0, 2, 1, 3))

    # weights: [g][tg][p][k][blk][c]
    w16 = Wc.astype(np.float16)
    wprep = np.ascontiguousarray(
        w16.reshape(G, K, K, NBLK, 128, W).transpose(0, 1, 4, 2, 3, 5))

    in_maps = []
    for c in range(NCORES):
        g0 = c * IMGS_PER_CORE
        in_maps.append({
            "fin": np.ascontiguousarray(
                fprep[g0:g0 + IMGS_PER_CORE].reshape(IMGS_PER_CORE, 128, F_FREE)),
            "win": np.ascontiguousarray(
                wprep[g0:g0 + IMGS_PER_CORE].reshape(IMGS_PER_CORE, K, 128, W_FREE)),
        })
    return in_maps


def kernel(frames, core, bias):
    global last_results
    from concourse.bass_utils import run_bass_kernel_spmd

    frames = np.asarray(frames, dtype=np.float32)
    core = np.asarray(core, dtype=np.float32)

    if "nc" not in _compiled:
        _compiled["nc"] = _build_nc()
    nc = _compiled["nc"]

    in_maps = _host_prep(frames, core)
    trace = os.environ.get("KC_TRACE") == "1"
    tmpdir = os.environ.get("KC_TRACE_DIR") or None
    if tmpdir:
        os.makedirs(tmpdir, exist_ok=True)
    res = run_bass_kernel_spmd(nc, in_maps, list(range(NCORES)), trace=trace,
                               tmpdir=tmpdir)
    last_results = res

    G = NCORES * IMGS_PER_CORE
    out = np.empty((G, H, W), np.float32)
    for c in range(NCORES):
        o = res.results[c]["oout"]  # [2, 128, 2048] f32
        for img in range(IMGS_PER_CORE):
            out[c * IMGS_PER_CORE + img] = (
                o[img].reshape(128, NBLK, W).transpose(1, 0, 2).reshape(H, W))
    return out.reshape(4, 4, H, W)



# revision 2
# speedup vs baseline: 2.0199x; 2.0199x over previous
"""Per-pixel adaptive 5x5 conv (KPN) for Trainium2, 8-core data parallel.

out[g,r,c] = sum_{i,j} core[g,5i+j,r,c] * frames_pad[g,r+i-2,c+j-2]
with g = flattened (B,N) = 16 image planes; 2 planes per NeuronCore.

Layout: partition p holds image rows 4p..4p+3 in the free dim
("rows-in-free"), so column taps (j) are free-dim offsets and row taps
(i) are merged on the TensorEngine with shift matrices:

  P_t[p,q,c] = W_t[p,q,c] * fin[p, par_j, q, joff_j + c]   (DVE fp16 2x)
  out[r]     = sum_t P_t[r + s_t],  s = i-2                (TensorE)

Host pre-shifts core rows by s so every P value needed outside [0,512)
lands on a zero-padded frame row -> contributes exactly 0; the 128x128
shift matrices (I / sub / super diagonal) truncate naturally at the
partition boundary. Per tap: 1 DVE mul + 4 matmuls (one per PSUM bank,
FD=512), accumulating in fp32 PSUM. ScalarE casts PSUM->SBUF fp16; the
store is fp16 with a host-side upcast. GpSimd does no elementwise work
(it shares its SBUF port with the DVE; running both slows each ~4x).
"""

import os
import sys

import numpy as np

for _p in ("/opt/trn_rl_repo",):
    if _p not in sys.path and os.path.isdir(_p):
        sys.path.insert(0, _p)

K = 5
NCORES = 8
IMGS_PER_CORE = 2
H = W = 512
QR = 4                      # image rows per partition
FCOLS = 518
F_FREE = 2 * QR * FCOLS     # 4144 (parity copies keep odd-j taps 4B-aligned)
TAPS = K * K
O_FREE = QR * W             # 2048

# i=2 group (s=0) first: its 4 matmuls initialize all 4 PSUM banks
I_ORDER = (2, 0, 1, 3, 4)
TAP_LIST = tuple((i, j) for i in I_ORDER for j in range(K))

_compiled = {}
last_results = None  # BassKernelResults of the most recent run (for test.py)


def _build_nc():
    import concourse.bacc as bacc
    import concourse.mybir as mybir
    from concourse.tile import TileContext

    f16 = mybir.dt.float16
    f32 = mybir.dt.float32

    nc = bacc.Bacc(None, target_bir_lowering=False, debug=False)
    fin = nc.dram_tensor("fin", [IMGS_PER_CORE, 128, F_FREE], f16,
                         kind="ExternalInput")
    win = nc.dram_tensor("win", [IMGS_PER_CORE, TAPS, 128, O_FREE], f16,
                         kind="ExternalInput")
    smat = nc.dram_tensor("smat", [128, 3 * 128], f16, kind="ExternalInput")
    oout = nc.dram_tensor("oout", [IMGS_PER_CORE, 128, O_FREE], f16,
                          kind="ExternalOutput")

    with TileContext(nc) as tc:
        with (
            tc.tile_pool(name="cpool", bufs=1) as cpool,
            tc.tile_pool(name="fpool", bufs=2) as fpool,
            tc.tile_pool(name="wpool", bufs=12) as wpool,
            tc.tile_pool(name="ppool", bufs=4) as ppool,
            tc.tile_pool(name="opool", bufs=2) as opool,
            tc.psum_pool(name="pspool", bufs=2) as pspool,
        ):
            sm_t = cpool.tile([128, 3 * 128], f16)
            nc.sync.dma_start(out=sm_t[:], in_=smat[:])
            sm = {"I": sm_t[:, 0:128], "P": sm_t[:, 128:256],
                  "M": sm_t[:, 256:384]}

            for img in range(IMGS_PER_CORE):
                f_t = fpool.tile([128, F_FREE], f16, tag="fin")
                nc.sync.dma_start(out=f_t[:], in_=fin[img])
                fv = f_t[:].rearrange("p (par q c) -> p par q c",
                                      par=2, q=QR, c=FCOLS)
                ps_t = pspool.tile([128, O_FREE], f32, tag="ps")

                for t, (i, j) in enumerate(TAP_LIST):
                    s = i - 2
                    par = j & 1
                    joff = j + par
                    w_t = wpool.tile([128, O_FREE], f16, tag="w")
                    nc.sync.dma_start(out=w_t[:], in_=win[img, t])
                    p_t = ppool.tile([128, O_FREE], f16, tag="p")
                    nc.vector.tensor_mul(
                        out=p_t[:].rearrange("p (q c) -> p q c", q=QR),
                        in0=w_t[:].rearrange("p (q c) -> p q c", q=QR),
                        in1=fv[:, par, :, joff:joff + W])
                    for q in range(QR):
                        qs = q + s
                        if 0 <= qs < QR:
                            lhsT, rblk = sm["I"], qs
                        elif qs >= QR:
                            lhsT, rblk = sm["P"], qs - QR
                        else:
                            lhsT, rblk = sm["M"], qs + QR
                        nc.tensor.matmul(
                            out=ps_t[:, q * W:(q + 1) * W],
                            lhsT=lhsT,
                            rhs=p_t[:, rblk * W:(rblk + 1) * W],
                            start=(t == 0),
                            stop=(t == TAPS - 1))

                o_t = opool.tile([128, O_FREE], f16, tag="o")
                nc.scalar.copy(out=o_t[:], in_=ps_t[:])
                nc.sync.dma_start(out=oout[img], in_=o_t[:])
    nc.finalize()
    return nc


def _host_prep(frames, core):
    """Build per-core in_maps. frames [4,4,1,512,512] f32, core [4,4,25,1,512,512]."""
    G = NCORES * IMGS_PER_CORE  # 16
    F = np.ascontiguousarray(frames.reshape(G, H, W))
    C = core.reshape(G, TAPS, H, W)

    Fc = np.pad(F, ((0, 0), (0, 0), (3, 4))).astype(np.float16)  # [G,512,519]
    fin = np.empty((G, 128, 2, QR, FCOLS), np.float16)
    for par in range(2):
        fin[:, :, par] = (Fc[:, :, (1 - par):(1 - par) + FCOLS]
                          .reshape(G, 128, QR, FCOLS))

    win = np.zeros((G, TAPS, H, W), np.float16)
    for t, (i, j) in enumerate(TAP_LIST):
        s = i - 2
        src = C[:, i * K + j]
        if s > 0:
            win[:, t, s:] = src[:, :H - s]
        elif s < 0:
            win[:, t, :s] = src[:, -s:]
        else:
            win[:, t] = src

    smat = np.concatenate([np.eye(128, dtype=np.float16),
                           np.eye(128, k=-1, dtype=np.float16),
                           np.eye(128, k=+1, dtype=np.float16)], axis=1)
    smat = np.ascontiguousarray(smat)

    fin = fin.reshape(G, 128, F_FREE)
    win = win.reshape(G, TAPS, 128, O_FREE)
    in_maps = []
    for c in range(NCORES):
        g0 = c * IMGS_PER_CORE
        in_maps.append({
            "fin": np.ascontiguousarray(fin[g0:g0 + IMGS_PER_CORE]),
            "win": np.ascontiguousarray(win[g0:g0 + IMGS_PER_CORE]),
            "smat": smat,
        })
    return in_maps


def kernel(frames, core, bias):
    global last_results
    from concourse.bass_utils import run_bass_kernel_spmd

    frames = np.asarray(frames, dtype=np.float32)
    core = np.asarray(core, dtype=np.float32)

    if "nc" not in _compiled:
        _compiled["nc"] = _build_nc()
    nc = _compiled["nc"]

    in_maps = _host_prep(frames, core)
    trace = os.environ.get("KC_TRACE") == "1"
    tmpdir = os.environ.get("KC_TRACE_DIR") or None
    if tmpdir:
        os.makedirs(tmpdir, exist_ok=True)
    res = run_bass_kernel_spmd(nc, in_maps, list(range(NCORES)), trace=trace,
                               tmpdir=tmpdir)
    last_results = res

    G = NCORES * IMGS_PER_CORE
    out = np.empty((G, H, W), np.float32)
    for c in range(NCORES):
        o = res.results[c]["oout"]  # [2, 128, 2048] f16
        for img in range(IMGS_PER_CORE):
            out[c * IMGS_PER_CORE + img] = (
                o[img].reshape(H, W).astype(np.float32))
    return out.reshape(4, 4, H, W)


# revision 5
# speedup vs baseline: 2.0743x; 1.0269x over previous
"""Per-pixel adaptive 5x5 conv (KPN) for Trainium2, 8-core data parallel.

out[g,r,c] = sum_{i,j} core[g,5i+j,r,c] * frames_pad[g,r+i-2,c+j-2]
with g = flattened (B,N) = 16 image planes; 2 planes per NeuronCore.

Layout: partition p holds image rows 4p..4p+3 in the free dim
("rows-in-free"), so column taps (j) are free-dim offsets and row taps
(i) are merged on the TensorEngine with shift matrices:

  P_t[p,q,c] = W_t[p,q,c] * fin[p, q, joff_j + c]          (DVE fp16 2x)
  out[r]     = sum_t P_t[r + s_t],  s = i-2                (TensorE)

Host pre-shifts core rows by s so every P value needed outside [0,512)
lands on a zero-padded frame row -> contributes exactly 0; the 128x128
shift matrices (I / sub / super diagonal) truncate naturally at the
partition boundary. Per tap: 1 DVE mul + 4 matmuls (one per PSUM bank,
FD=512), accumulating in fp32 PSUM. GpSimd does no elementwise work
(it shares its SBUF port with the DVE; running both slows each ~4x).

DMA-stream discipline (the kernel sits at the DVE/HBM ridge, so every
HBM byte ahead of weights starves the DVE):
- only the parity-0 frame copy comes from HBM; the parity-1 copy (for
  odd-j taps' 4B alignment) is made on-chip by a 2-byte-shifted
  SBUF->SBUF DMA, off the HBM path;
- the first tap is quartered into 4 column-block products so the DVE
  starts as soon as ~0.7MB has landed;
- img1's frames are fetched mid-img0 where the weight-pool backlog
  absorbs the insertion;
- outputs are fp16 (host upcasts); final image drains PSUM with
  ScalarE and DVE halves in parallel; output DMAs ride the scalar and
  sync HWDGE rings.
"""

import os
import sys

import numpy as np

for _p in ("/opt/trn_rl_repo",):
    if _p not in sys.path and os.path.isdir(_p):
        sys.path.insert(0, _p)

K = 5
NCORES = 8
IMGS_PER_CORE = 2
H = W = 512
QR = 4                      # image rows per partition
FCOLS = 518
PAR_FREE = QR * FCOLS       # 2072 per parity copy
TAPS = K * K
O_FREE = QR * W             # 2048

# i=2 group (s=0) first so the first matmuls initialize all PSUM banks;
# within each group even-parity taps first so the par1 on-chip copy can
# finish a few taps late.
I_ORDER = (2, 0, 1, 3, 4)
J_ORDER = (0, 2, 4, 1, 3)
TAP_LIST = tuple((i, j) for i in I_ORDER for j in J_ORDER)

_compiled = {}
last_results = None  # BassKernelResults of the most recent run (for test.py)


def _build_nc():
    import concourse.bacc as bacc
    import concourse.mybir as mybir
    from concourse.tile import TileContext

    f16 = mybir.dt.float16
    f32 = mybir.dt.float32

    nc = bacc.Bacc(None, target_bir_lowering=False, debug=False)
    fin = nc.dram_tensor("fin", [IMGS_PER_CORE, 128, PAR_FREE], f16,
                         kind="ExternalInput")
    win = nc.dram_tensor("win", [IMGS_PER_CORE, TAPS, 128, O_FREE], f16,
                         kind="ExternalInput")
    smat = nc.dram_tensor("smat", [128, 3 * 128], f16, kind="ExternalInput")
    oout = nc.dram_tensor("oout", [IMGS_PER_CORE, 128, O_FREE], f16,
                          kind="ExternalOutput")

    with TileContext(nc) as tc:
        with (
            tc.tile_pool(name="cpool", bufs=1) as cpool,
            tc.tile_pool(name="fpool", bufs=4) as fpool,
            tc.tile_pool(name="wpool", bufs=14) as wpool,
            tc.tile_pool(name="wqpool", bufs=4) as wqpool,
            tc.tile_pool(name="ppool", bufs=6) as ppool,
            tc.tile_pool(name="pqpool", bufs=4) as pqpool,
            tc.tile_pool(name="opool", bufs=2) as opool,
            tc.psum_pool(name="pspool", bufs=2) as pspool,
        ):
            # --- ramp: smallest, most urgent transfers first ---
            wq_ts = []
            for q in range(QR):
                wq = wqpool.tile([128, W], f16, name=f"wq{q}", tag=f"wq{q}")
                nc.sync.dma_start(out=wq[:],
                                  in_=win[0, 0][:, q * W:(q + 1) * W])
                wq_ts.append(wq)

            f_ts = [[None, None] for _ in range(IMGS_PER_CORE)]

            def fin_dma(img):
                # parity 0 from HBM; parity 1 = same data shifted one
                # column, built by an on-chip SBUF->SBUF DMA (col 0 of
                # the par1 view is never read: odd-j taps have joff>=2).
                t0 = fpool.tile([128, PAR_FREE], f16, name=f"f{img}0",
                                tag=f"f{img}0")
                nc.sync.dma_start(out=t0[:], in_=fin[img])
                t1 = fpool.tile([128, PAR_FREE], f16, name=f"f{img}1",
                                tag=f"f{img}1")
                v0 = t0[:].rearrange("p (q c) -> p q c", q=QR)
                v1 = t1[:].rearrange("p (q c) -> p q c", q=QR)
                nc.sync.dma_start(out=v1[:, :, 1:FCOLS],
                                  in_=v0[:, :, 0:FCOLS - 1])
                f_ts[img] = [t0, t1]

            fin_dma(0)
            sm_t = cpool.tile([128, 3 * 128], f16)
            nc.sync.dma_start(out=sm_t[:], in_=smat[:])
            sm = {"I": sm_t[:, 0:128], "P": sm_t[:, 128:256],
                  "M": sm_t[:, 256:384]}

            def fview(img, par):
                return f_ts[img][par][:].rearrange("p (q c) -> p q c", q=QR)

            # --- img0 tap0 (i=2, j=0; s=0), quartered for fast start ---
            ps_ts = [None, None]
            ps_ts[0] = pspool.tile([128, O_FREE], f32, name="ps", tag="ps")
            for q in range(QR):
                pq = pqpool.tile([128, W], f16, name=f"pq{q}", tag=f"pq{q}")
                nc.vector.tensor_mul(out=pq[:], in0=wq_ts[q][:],
                                     in1=fview(0, 0)[:, q, 0:W])
                nc.tensor.matmul(out=ps_ts[0][:, q * W:(q + 1) * W],
                                 lhsT=sm["I"], rhs=pq[:],
                                 start=True, stop=False)

            # --- main tap stream ---
            for img in range(IMGS_PER_CORE):
                if img > 0:
                    ps_ts[img] = pspool.tile([128, O_FREE], f32, name="ps",
                                             tag="ps")
                ps_t = ps_ts[img]
                for t, (i, j) in enumerate(TAP_LIST):
                    if img == 0 and t == 0:
                        continue  # done above
                    s = i - 2
                    par = j & 1
                    joff = j + par
                    w_t = wpool.tile([128, O_FREE], f16, name="w", tag="w")
                    nc.sync.dma_start(out=w_t[:], in_=win[img, t])
                    if img == 0 and t == 14:
                        fin_dma(1)  # mid-stream, absorbed by wpool backlog
                    first = (img > 0 and t == 0)
                    p_t = ppool.tile([128, O_FREE], f16, name="p", tag="p")
                    nc.vector.tensor_mul(
                        out=p_t[:].rearrange("p (q c) -> p q c", q=QR),
                        in0=w_t[:].rearrange("p (q c) -> p q c", q=QR),
                        in1=fview(img, par)[:, :, joff:joff + W])
                    for q in range(QR):
                        qs = q + s
                        if 0 <= qs < QR:
                            lhsT, rblk = sm["I"], qs
                        elif qs >= QR:
                            lhsT, rblk = sm["P"], qs - QR
                        else:
                            lhsT, rblk = sm["M"], qs + QR
                        nc.tensor.matmul(
                            out=ps_t[:, q * W:(q + 1) * W],
                            lhsT=lhsT,
                            rhs=p_t[:, rblk * W:(rblk + 1) * W],
                            start=first,
                            stop=(t == TAPS - 1))

                # drain: img0 on ScalarE (DVE is busy); final image on
                # ScalarE + DVE halves in parallel.
                o_t = opool.tile([128, O_FREE], f16, name="o", tag="o")
                HALF = O_FREE // 2
                lo, hi = slice(0, HALF), slice(HALF, O_FREE)
                last = img == IMGS_PER_CORE - 1
                nc.scalar.copy(out=o_t[:, lo], in_=ps_t[:, lo])
                nc.scalar.dma_start(out=oout[img][:, lo], in_=o_t[:, lo])
                if last:
                    nc.vector.tensor_copy(o_t[:, hi], ps_t[:, hi])
                    nc.sync.dma_start(out=oout[img][:, hi], in_=o_t[:, hi])
                else:
                    nc.scalar.copy(out=o_t[:, hi], in_=ps_t[:, hi])
                    nc.scalar.dma_start(out=oout[img][:, hi], in_=o_t[:, hi])
    nc.finalize()
    return nc


def _host_prep(frames, core):
    """Build per-core in_maps. frames [4,4,1,512,512] f32, core [4,4,25,1,512,512]."""
    G = NCORES * IMGS_PER_CORE  # 16
    F = np.ascontiguousarray(frames.reshape(G, H, W))
    C = core.reshape(G, TAPS, H, W)

    # parity-0 copy only: fin[p, q, cc] = Fc[4p+q, 1+cc]
    Fc = np.pad(F, ((0, 0), (0, 0), (3, 4))).astype(np.float16)  # [G,512,519]
    fin = np.ascontiguousarray(
        Fc[:, :, 1:1 + FCOLS].reshape(G, 128, QR * FCOLS))

    win = np.zeros((G, TAPS, H, W), np.float16)
    for t, (i, j) in enumerate(TAP_LIST):
        s = i - 2
        src = C[:, i * K + j]
        if s > 0:
            win[:, t, s:] = src[:, :H - s]
        elif s < 0:
            win[:, t, :s] = src[:, -s:]
        else:
            win[:, t] = src

    smat = np.concatenate([np.eye(128, dtype=np.float16),
                           np.eye(128, k=-1, dtype=np.float16),
                           np.eye(128, k=+1, dtype=np.float16)], axis=1)
    smat = np.ascontiguousarray(smat)

    win = win.reshape(G, TAPS, 128, O_FREE)
    in_maps = []
    for c in range(NCORES):
        g0 = c * IMGS_PER_CORE
        in_maps.append({
            "fin": np.ascontiguousarray(fin[g0:g0 + IMGS_PER_CORE]),
            "win": np.ascontiguousarray(win[g0:g0 + IMGS_PER_CORE]),
            "smat": smat,
        })
    return in_maps


def kernel(frames, core, bias):
    global last_results
    from concourse.bass_utils import run_bass_kernel_spmd

    frames = np.asarray(frames, dtype=np.float32)
    core = np.asarray(core, dtype=np.float32)

    if "nc" not in _compiled:
        _compiled["nc"] = _build_nc()
    nc = _compiled["nc"]

    in_maps = _host_prep(frames, core)
    trace = os.environ.get("KC_TRACE") == "1"
    tmpdir = os.environ.get("KC_TRACE_DIR") or None
    if tmpdir:
        os.makedirs(tmpdir, exist_ok=True)
    res = run_bass_kernel_spmd(nc, in_maps, list(range(NCORES)), trace=trace,
                               tmpdir=tmpdir)
    last_results = res

    G = NCORES * IMGS_PER_CORE
    out = np.empty((G, H, W), np.float32)
    for c in range(NCORES):
        o = res.results[c]["oout"]  # [2, 128, 2048] f16
        for img in range(IMGS_PER_CORE):
            out[c * IMGS_PER_CORE + img] = (
                o[img].reshape(H, W).astype(np.float32))
    return out.reshape(4, 4, H, W)


# revision 6
# speedup vs baseline: 2.1152x; 1.0197x over previous
"""Per-pixel adaptive 5x5 conv (KPN) for Trainium2, 8-core data parallel.

out[g,r,c] = sum_{i,j} core[g,5i+j,r,c] * frames_pad[g,r+i-2,c+j-2]
with g = flattened (B,N) = 16 image planes; 2 planes per NeuronCore.

Layout: partition p holds image rows 4p..4p+3 in the free dim
("rows-in-free"), so column taps (j) are free-dim offsets and row taps
(i) are merged on the TensorEngine with shift matrices:

  P_t[p,q,c] = W_t[p,q,c] * fin[p, q, joff_j + c]          (DVE fp16 2x)
  out[r]     = sum_t P_t[r + s_t],  s = i-2                (TensorE)

Host pre-shifts core rows by s so every P value needed outside [0,512)
lands on a zero-padded frame row -> contributes exactly 0; the 128x128
shift matrices (I / sub / super diagonal) truncate naturally at the
partition boundary. Per tap: 1 DVE mul + 4 matmuls (one per PSUM bank,
FD=512), accumulating in fp32 PSUM. GpSimd does no elementwise work
(it shares its SBUF port with the DVE; running both slows each ~4x).

DMA-stream discipline (the kernel sits at the DVE/HBM ridge, so every
HBM byte ahead of weights starves the DVE):
- only the parity-0 frame copy comes from HBM; the parity-1 copy (for
  odd-j taps' 4B alignment) is made on-chip by a 2-byte-shifted
  SBUF->SBUF DMA, off the HBM path;
- the first tap is quartered into 4 column-block products so the DVE
  starts as soon as ~0.7MB has landed;
- img1's frames are fetched mid-img0 where the weight-pool backlog
  absorbs the insertion;
- outputs are fp16 (host upcasts); final image drains PSUM with
  ScalarE and DVE halves in parallel; output DMAs ride the scalar and
  sync HWDGE rings.
"""

import os
import sys

import numpy as np

for _p in ("/opt/trn_rl_repo",):
    if _p not in sys.path and os.path.isdir(_p):
        sys.path.insert(0, _p)

K = 5
NCORES = 8
IMGS_PER_CORE = 2
H = W = 512
QR = 4                      # image rows per partition
FCOLS = 518
PAR_FREE = QR * FCOLS       # 2072 per parity copy
TAPS = K * K
O_FREE = QR * W             # 2048

# i=2 group (s=0) first so the first matmuls initialize all PSUM banks;
# within each group even-parity taps first so the par1 on-chip copy can
# finish a few taps late.
I_ORDER = (2, 0, 1, 3, 4)
J_ORDER = (0, 2, 4, 1, 3)
TAP_LIST = tuple((i, j) for i in I_ORDER for j in J_ORDER)

_compiled = {}
last_results = None  # BassKernelResults of the most recent run (for test.py)


def _build_nc():
    import concourse.bacc as bacc
    import concourse.mybir as mybir
    from concourse.tile import TileContext

    f16 = mybir.dt.float16
    f32 = mybir.dt.float32

    nc = bacc.Bacc(None, target_bir_lowering=False, debug=False)
    fin = nc.dram_tensor("fin", [IMGS_PER_CORE, 128, PAR_FREE], f16,
                         kind="ExternalInput")
    win = nc.dram_tensor("win", [IMGS_PER_CORE, TAPS, 128, O_FREE], f16,
                         kind="ExternalInput")
    winq = nc.dram_tensor("winq", [QR, 128, W], f16, kind="ExternalInput")
    smat = nc.dram_tensor("smat", [128, 3 * 128], f16, kind="ExternalInput")
    oout = nc.dram_tensor("oout", [IMGS_PER_CORE, 128, O_FREE], f16,
                          kind="ExternalOutput")

    with TileContext(nc) as tc:
        with (
            tc.tile_pool(name="cpool", bufs=1) as cpool,
            tc.tile_pool(name="fpool", bufs=4) as fpool,
            tc.tile_pool(name="wpool", bufs=14) as wpool,
            tc.tile_pool(name="wqpool", bufs=4) as wqpool,
            tc.tile_pool(name="ppool", bufs=6) as ppool,
            tc.tile_pool(name="pqpool", bufs=4) as pqpool,
            tc.tile_pool(name="opool", bufs=2) as opool,
            tc.psum_pool(name="pspool", bufs=2) as pspool,
        ):
            # --- ramp: smallest, most urgent transfers first ---
            wq_ts = []
            for q in range(QR):
                wq = wqpool.tile([128, W], f16, name=f"wq{q}", tag=f"wq{q}")
                nc.sync.dma_start(out=wq[:], in_=winq[q])
                wq_ts.append(wq)

            f_ts = [[None, None] for _ in range(IMGS_PER_CORE)]

            def fin_dma(img):
                # parity 0 from HBM; parity 1 = same data shifted one
                # column, built by an on-chip SBUF->SBUF DMA (col 0 of
                # the par1 view is never read: odd-j taps have joff>=2).
                t0 = fpool.tile([128, PAR_FREE], f16, name=f"f{img}0",
                                tag=f"f{img}0")
                nc.sync.dma_start(out=t0[:], in_=fin[img])
                t1 = fpool.tile([128, PAR_FREE], f16, name=f"f{img}1",
                                tag=f"f{img}1")
                v0 = t0[:].rearrange("p (q c) -> p q c", q=QR)
                v1 = t1[:].rearrange("p (q c) -> p q c", q=QR)
                nc.scalar.dma_start(out=v1[:, :, 1:FCOLS],
                                    in_=v0[:, :, 0:FCOLS - 1])
                f_ts[img] = [t0, t1]

            fin_dma(0)
            sm_t = cpool.tile([128, 3 * 128], f16)
            nc.scalar.dma_start(out=sm_t[:], in_=smat[:])
            sm = {"I": sm_t[:, 0:128], "P": sm_t[:, 128:256],
                  "M": sm_t[:, 256:384]}

            def fview(img, par):
                return f_ts[img][par][:].rearrange("p (q c) -> p q c", q=QR)

            # --- img0 tap0 (i=2, j=0; s=0), quartered for fast start ---
            ps_ts = [None, None]
            ps_ts[0] = pspool.tile([128, O_FREE], f32, name="ps", tag="ps")
            for q in range(QR):
                pq = pqpool.tile([128, W], f16, name=f"pq{q}", tag=f"pq{q}")
                nc.vector.tensor_mul(out=pq[:], in0=wq_ts[q][:],
                                     in1=fview(0, 0)[:, q, 0:W])
                nc.tensor.matmul(out=ps_ts[0][:, q * W:(q + 1) * W],
                                 lhsT=sm["I"], rhs=pq[:],
                                 start=True, stop=False)

            # --- main tap stream ---
            for img in range(IMGS_PER_CORE):
                if img > 0:
                    ps_ts[img] = pspool.tile([128, O_FREE], f32, name="ps",
                                             tag="ps")
                ps_t = ps_ts[img]
                for t, (i, j) in enumerate(TAP_LIST):
                    if img == 0 and t == 0:
                        continue  # done above
                    s = i - 2
                    par = j & 1
                    joff = j + par
                    w_t = wpool.tile([128, O_FREE], f16, name="w", tag="w")
                    nc.sync.dma_start(out=w_t[:], in_=win[img, t])
                    if img == 0 and t == 14:
                        fin_dma(1)  # mid-stream, absorbed by wpool backlog
                    first = (img > 0 and t == 0)
                    p_t = ppool.tile([128, O_FREE], f16, name="p", tag="p")
                    nc.vector.tensor_mul(
                        out=p_t[:].rearrange("p (q c) -> p q c", q=QR),
                        in0=w_t[:].rearrange("p (q c) -> p q c", q=QR),
                        in1=fview(img, par)[:, :, joff:joff + W])
                    for q in range(QR):
                        qs = q + s
                        if 0 <= qs < QR:
                            lhsT, rblk = sm["I"], qs
                        elif qs >= QR:
                            lhsT, rblk = sm["P"], qs - QR
                        else:
                            lhsT, rblk = sm["M"], qs + QR
                        nc.tensor.matmul(
                            out=ps_t[:, q * W:(q + 1) * W],
                            lhsT=lhsT,
                            rhs=p_t[:, rblk * W:(rblk + 1) * W],
                            start=first,
                            stop=(t == TAPS - 1))

                # drain: img0 on ScalarE (DVE is busy); final image on
                # ScalarE + DVE halves in parallel.
                HALF = O_FREE // 2
                lo, hi = slice(0, HALF), slice(HALF, O_FREE)
                last = img == IMGS_PER_CORE - 1
                o_lo = opool.tile([128, HALF], f16, name="olo", tag="olo")
                o_hi = opool.tile([128, HALF], f16, name="ohi", tag="ohi")
                nc.scalar.copy(out=o_lo[:], in_=ps_t[:, lo])
                nc.scalar.dma_start(out=oout[img][:, lo], in_=o_lo[:])
                if last:
                    nc.vector.tensor_copy(o_hi[:], ps_t[:, hi])
                    nc.sync.dma_start(out=oout[img][:, hi], in_=o_hi[:])
                else:
                    nc.scalar.copy(out=o_hi[:], in_=ps_t[:, hi])
                    nc.scalar.dma_start(out=oout[img][:, hi], in_=o_hi[:])
    nc.finalize()
    return nc


def _host_prep(frames, core):
    """Build per-core in_maps. frames [4,4,1,512,512] f32, core [4,4,25,1,512,512]."""
    G = NCORES * IMGS_PER_CORE  # 16
    F = np.ascontiguousarray(frames.reshape(G, H, W))
    C = core.reshape(G, TAPS, H, W)

    # parity-0 copy only: fin[p, q, cc] = Fc[4p+q, 1+cc]
    Fc = np.pad(F, ((0, 0), (0, 0), (3, 4))).astype(np.float16)  # [G,512,519]
    fin = np.ascontiguousarray(
        Fc[:, :, 1:1 + FCOLS].reshape(G, 128, QR * FCOLS))

    win = np.zeros((G, TAPS, H, W), np.float16)
    for t, (i, j) in enumerate(TAP_LIST):
        s = i - 2
        src = C[:, i * K + j]
        if s > 0:
            win[:, t, s:] = src[:, :H - s]
        elif s < 0:
            win[:, t, :s] = src[:, -s:]
        else:
            win[:, t] = src

    smat = np.concatenate([np.eye(128, dtype=np.float16),
                           np.eye(128, k=-1, dtype=np.float16),
                           np.eye(128, k=+1, dtype=np.float16)], axis=1)
    smat = np.ascontiguousarray(smat)

    win = win.reshape(G, TAPS, 128, O_FREE)
    # contiguous column-block quarters of img0's first tap, per core
    in_maps = []
    for c in range(NCORES):
        g0 = c * IMGS_PER_CORE
        winq = np.ascontiguousarray(
            win[g0, 0].reshape(128, QR, W).transpose(1, 0, 2))
        in_maps.append({
            "fin": np.ascontiguousarray(fin[g0:g0 + IMGS_PER_CORE]),
            "win": np.ascontiguousarray(win[g0:g0 + IMGS_PER_CORE]),
            "winq": winq,
            "smat": smat,
        })
    return in_maps


def kernel(frames, core, bias):
    global last_results
    from concourse.bass_utils import run_bass_kernel_spmd

    frames = np.asarray(frames, dtype=np.float32)
    core = np.asarray(core, dtype=np.float32)

    if "nc" not in _compiled:
        _compiled["nc"] = _build_nc()
    nc = _compiled["nc"]

    in_maps = _host_prep(frames, core)
    trace = os.environ.get("KC_TRACE") == "1"
    tmpdir = os.environ.get("KC_TRACE_DIR") or None
    if tmpdir:
        os.makedirs(tmpdir, exist_ok=True)
    res = run_bass_kernel_spmd(nc, in_maps, list(range(NCORES)), trace=trace,
                               tmpdir=tmpdir)
    last_results = res

    G = NCORES * IMGS_PER_CORE
    out = np.empty((G, H, W), np.float32)
    for c in range(NCORES):
        o = res.results[c]["oout"]  # [2, 128, 2048] f16
        for img in range(IMGS_PER_CORE):
            out[c * IMGS_PER_CORE + img] = (
                o[img].reshape(H, W).astype(np.float32))
    return out.reshape(4, 4, H, W)
